# revision 26
# baseline (speedup 1.0000x reference)
"""AttentionBlock (GroupNorm + single-head self-attention + proj + residual) on 8 Trainium2
NeuronCores, data-parallel over the batch (16 samples -> 2 per core).

Host-side algebra folds the four weight matrices into two (exact, before fp8
quantization -- softmax over j is invariant to terms constant in j):
  S[i,j]   = (Wq h_i + bq).(Wk h_j + bk)  ~softmax~  h_j . (M^T h_i + r)
             with M = Wq^T Wk,  r = Wk^T bq          (bk term cancels)
  proj(Av) = A (Wpv h) + c0,  Wpv = Wp Wv,  c0 = Wp bv + pb  (sum_j A = 1)

Per-sample device math (C=512 channels, N=1024 tokens):
  h   = GroupNorm(x; 8 groups) * w + b           [C, N]
  u   = M^T h + r                                [C, N]   (c on partitions)
  v'T = h^T Wpv^T                                [N, C]   (tokens on partitions)
  S^T = h^T u * scale                            [j, i]   (j on partitions)
  E   = exp(S^T - 1)   den[i] = sum_j E[j,i]     (den via ones-matmuls, i on partitions)
  oT[i,c] = sum_j E[j,i] v'T[j,c];  oT *= 1/den[i]
  y   = (x + c0) + transpose(oT)                 [C, N]

All big matmuls run in fp8e4 (e4m3) with MatmulPerfMode.DoubleRow (2 k-tiles
per instruction, 157 TF/s = 2x bf16).  Weights are pre-scaled x16 on the host
so fp8 stays out of subnormals; the scale is folded into the exp() argument
(1/16) and the final residual (1/16).  x and y travel as bf16 to halve DMA.
"""

import numpy as np
import ml_dtypes

import concourse.bacc as bacc
import concourse.tile as tile
from concourse import mybir
from concourse.bass_utils import run_bass_kernel_spmd
from concourse.hw_specs import get_activation_tables as _gat
from concourse.masks import make_identity

F32 = mybir.dt.float32
BF16 = mybir.dt.bfloat16
F8 = mybir.dt.float8e4
AF = mybir.ActivationFunctionType
OP = mybir.AluOpType
DR = mybir.MatmulPerfMode.DoubleRow

NCORES = 8
S = 2          # samples per core
C = 512
N = 1024       # H*W
CT = C // 128  # channel tiles
NT = N // 128  # token tiles
EPS = 1e-5
SCALE = float(C) ** -0.5
WS = 16.0          # host-side weight scale (keeps fp8 weights out of subnormals)
INV_WS = 1.0 / WS

_ONE_SET = "natural_log_exp_and_others"


def _gat_filtered(arch):
    return {name: (fns if name == _ONE_SET else set())
            for name, fns in _gat(arch).items()}


bacc.get_activation_tables = _gat_filtered


def build_nc():
    nc = bacc.Bacc("TRN2", target_bir_lowering=False)
    x_d = nc.dram_tensor("x", [S, C, N], BF16, kind="ExternalInput")
    sm_d = nc.dram_tensor("smat", [C, C], F8, kind="ExternalInput")
    vw_d = nc.dram_tensor("vwT", [C, C], F8, kind="ExternalInput")
    nw_d = nc.dram_tensor("norm_w", [C], F32, kind="ExternalInput")
    nb_d = nc.dram_tensor("norm_b", [C], F32, kind="ExternalInput")
    ub_d = nc.dram_tensor("ub", [C], F32, kind="ExternalInput")
    c0_d = nc.dram_tensor("c0", [C], F32, kind="ExternalInput")
    gm_d = nc.dram_tensor("gmat", [128, 128], F32, kind="ExternalInput")
    out_d = nc.dram_tensor("out", [S, C, N], BF16, kind="ExternalOutput")

    with tile.TileContext(nc) as tc:
        with (
            tc.tile_pool(name="consts", bufs=1) as consts,
            tc.tile_pool(name="xp", bufs=2) as xp,
            tc.tile_pool(name="hp", bufs=2) as hp,
            tc.tile_pool(name="up", bufs=2) as up,
            tc.tile_pool(name="vp", bufs=2) as vp,
            tc.tile_pool(name="esp", bufs=2) as esp,
            tc.tile_pool(name="otp", bufs=2) as otp,
            tc.tile_pool(name="finp", bufs=2) as finp,
            tc.tile_pool(name="statp", bufs=4) as statp,
            tc.tile_pool(name="ps_big", bufs=2, space="PSUM") as ps_big,
            tc.tile_pool(name="ps_mid", bufs=2, space="PSUM") as ps_mid,
            tc.tile_pool(name="ps_sm", bufs=1, space="PSUM") as ps_sm,
        ):
            x_sb, h_sb, u_sb, vT_sb = {}, {}, {}, {}
            es_sb, oT_sb, fin_sb, recip = {}, {}, {}, {}
            ident_bf = consts.tile([128, 128], BF16, tag="identbf")
            make_identity(nc, ident_bf)
            ones8 = consts.tile([128, 2, 1], F8, tag="ones")
            nc.vector.memset(ones8, 1.0)
            epsb = consts.tile([128, 1], F32, tag="eps")
            nc.vector.memset(epsb, EPS)
            neg1 = consts.tile([128, 1], F32, tag="neg1")
            nc.vector.memset(neg1, -1.0)
            # warm the ACT table set under the DMA shadow
            warm = statp.tile([128, 1], F32, tag="tmp", name="warm")
            nc.scalar.activation(warm, epsb, AF.Exp, bias=0.0, scale=1.0)
            for s in range(S):
                x_sb[s] = xp.tile([128, CT, N], BF16, tag="x", name=f"x{s}")
            # small consts on the gpsimd queue (parallel with sync's x stream)
            gmat = consts.tile([128, 128], F32, tag="gmat")
            nc.gpsimd.dma_start(gmat, gm_d.ap())
            nw = consts.tile([128, CT], F32, tag="nw")
            nc.gpsimd.dma_start(nw, nw_d.ap().rearrange("(t p) -> p t", p=128))
            nb = consts.tile([128, CT], F32, tag="nb")
            nc.gpsimd.dma_start(nb, nb_d.ap().rearrange("(t p) -> p t", p=128))
            ub = consts.tile([128, CT], F32, tag="ub")
            nc.gpsimd.dma_start(ub, ub_d.ap().rearrange("(t p) -> p t", p=128))
            c0 = consts.tile([128, CT], F32, tag="c0")
            nc.gpsimd.dma_start(c0, c0_d.ap().rearrange("(t p) -> p t", p=128))
            # weights ride the gpsimd software queue; sync+scalar hwdge queues
            # stream x(s0) in fine chunks so bn_stats can chase the DMA
            smat = consts.tile([128, CT, C], F8, tag="smat")
            sm_r = sm_d.ap().rearrange("(kc p) o -> p kc o", p=128)
            for kc in range(CT):
                nc.gpsimd.dma_start(smat[:, kc, :], sm_r[:, kc, :])
            vwT = consts.tile([128, CT, C], F8, tag="vwT")
            vw_r = vw_d.ap().rearrange("(kc p) o -> p kc o", p=128)
            for kc in range(CT):
                nc.gpsimd.dma_start(vwT[:, kc, :], vw_r[:, kc, :])
            for ct in range(CT):
                for hh in range(2):
                    eng = nc.sync if hh == 0 else nc.scalar
                    eng.dma_start(x_sb[0][:, ct, hh * 512:(hh + 1) * 512],
                                  x_d[0, ct * 128:(ct + 1) * 128, hh * 512:(hh + 1) * 512])
            for ct in range(CT):
                eng = nc.sync if ct % 2 == 0 else nc.scalar
                eng.dma_start(x_sb[1][:, ct, :], x_d[1, ct * 128:(ct + 1) * 128, :])
            # dependency-free transposes ramp the PE clock out of its low
            # pstate while the x DMA streams in
            for w in range(24):
                wps = ps_big.tile([128, N], BF16, tag="big", name=f"wps{w % 4}")
                nc.tensor.transpose(wps[:, 0:128], ident_bf, ident_bf)

            def emit_gn(s):
                h_sb[s] = hp.tile([128, CT, N], F8, tag="h", name=f"h{s}")
                # per-partition (mean, E[x^2]) for all 4 c-tiles: mv[:, ct, 0:2]
                mv = statp.tile([128, CT, 2], F32, tag="mv", name=f"mv{s}")
                for ct in range(CT):
                    st = statp.tile([128, 2, 6], F32, tag="bnst")
                    for i in range(2):
                        nc.vector.bn_stats(st[:, i, :], x_sb[s][:, ct, i * 512:(i + 1) * 512])
                    nc.vector.bn_aggr(mv[:, ct, :], st)
                # E[x^2] = var + mean^2 (batched over all c-tiles)
                msq = statp.tile([128, CT, 2], F32, tag="msq", name=f"msq{s}")
                nc.vector.tensor_copy(msq[:, :, 0], mv[:, :, 0])
                nc.vector.tensor_tensor(msq[:, :, 1], mv[:, :, 0], mv[:, :, 0], OP.mult)
                nc.vector.tensor_tensor(msq[:, :, 1], msq[:, :, 1], mv[:, :, 1], OP.add)
                # group-average + broadcast back to all partitions: ONE matmul
                gps = ps_sm.tile([128, 2 * CT], F32, tag="gnagg", name=f"gps{s}")
                nc.tensor.matmul(gps[:, 0:2 * CT], lhsT=gmat,
                                 rhs=msq.rearrange("p a b -> p (a b)"),
                                 start=True, stop=True)
                gst = statp.tile([128, CT, 2], F32, tag="gst", name=f"gst{s}")
                nc.vector.tensor_copy(gst.rearrange("p a b -> p (a b)"), gps[:, 0:2 * CT])
                # scale = rstd * w ; shift = b - mean * scale
                sc = statp.tile([128, CT, 2], F32, tag="sc", name=f"sc{s}")
                tmp = statp.tile([128, CT], F32, tag="tmp", name=f"tmp{s}")
                nc.vector.tensor_tensor(tmp, gst[:, :, 0], gst[:, :, 0], OP.mult)
                nc.vector.tensor_tensor(tmp, gst[:, :, 1], tmp, OP.subtract)  # var
                # rstd = exp(-0.5*ln(var+eps)); Ln+Exp live in one ACT table set
                nc.scalar.activation(tmp, tmp, AF.Ln, bias=epsb, scale=1.0)
                nc.scalar.activation(tmp, tmp, AF.Exp, bias=0.0, scale=-0.5)
                nc.vector.tensor_tensor(sc[:, :, 0], tmp, nw, OP.mult)
                nc.vector.tensor_tensor(tmp, gst[:, :, 0], sc[:, :, 0], OP.mult)
                nc.vector.tensor_tensor(sc[:, :, 1], nb, tmp, OP.subtract)
                for ct in range(CT):
                    if ct < 2:
                        nc.scalar.activation(h_sb[s][:, ct, :], x_sb[s][:, ct, :],
                                             AF.Identity, bias=sc[:, ct, 1:2],
                                             scale=sc[:, ct, 0:1])
                    else:
                        nc.vector.tensor_scalar(h_sb[s][:, ct, :], x_sb[s][:, ct, :],
                                                sc[:, ct, 0:1], sc[:, ct, 1:2],
                                                OP.mult, OP.add)

            # ---------------- u = M^T h + r ; v'T = h^T Wpv^T ----------------
            def emit_uv(s):
                u_sb[s] = up.tile([128, CT, N], F8, tag="u", name=f"u{s}")
                vT_sb[s] = vp.tile([128, NT, C], F8, tag="vT", name=f"vT{s}")
                for mo in range(CT):
                    ps = ps_big.tile([128, N], F32, tag="big")
                    for nch in range(2):
                        for kp2 in range(CT // 2):
                            nc.tensor.matmul(
                                ps[:, nch * 512:(nch + 1) * 512],
                                lhsT=smat[:, 2 * kp2:2 * kp2 + 2,
                                          mo * 128:(mo + 1) * 128],
                                rhs=h_sb[s][:, 2 * kp2:2 * kp2 + 2,
                                            nch * 512:(nch + 1) * 512],
                                start=(kp2 == 0), stop=(kp2 == CT // 2 - 1),
                                perf_mode=DR)
                    nc.scalar.activation(u_sb[s][:, mo, :], ps, AF.Identity,
                                         bias=ub[:, mo:mo + 1], scale=1.0)
                for it in range(NT):
                    ps = ps_mid.tile([128, 512], F32, tag="mid")
                    for kp2 in range(CT // 2):
                        nc.tensor.matmul(ps,
                                         lhsT=h_sb[s][:, 2 * kp2:2 * kp2 + 2,
                                                      it * 128:(it + 1) * 128],
                                         rhs=vwT[:, 2 * kp2:2 * kp2 + 2, :],
                                         start=(kp2 == 0), stop=(kp2 == CT // 2 - 1),
                                         perf_mode=DR)
                    nc.vector.tensor_copy(vT_sb[s][:, it, :], ps)

            # ---------------- S^T = h^T u, exp, den ----------------
            def emit_s(s):
                es_sb[s] = esp.tile([128, NT, N], F8, tag="es", name=f"es{s}")
                den_ps = ps_sm.tile([128, NT], F32, tag="den", name=f"den{s}")
                nc.vector.memset(den_ps, 0.0)
                for jt in range(NT):
                    ps = ps_big.tile([128, N], F32, tag="big")
                    for nch in range(2):
                        for kp2 in range(CT // 2):
                            nc.tensor.matmul(
                                ps[:, nch * 512:(nch + 1) * 512],
                                lhsT=h_sb[s][:, 2 * kp2:2 * kp2 + 2,
                                             jt * 128:(jt + 1) * 128],
                                rhs=u_sb[s][:, 2 * kp2:2 * kp2 + 2,
                                            nch * 512:(nch + 1) * 512],
                                start=(kp2 == 0), stop=(kp2 == CT // 2 - 1),
                                perf_mode=DR)
                    # exp(S*scale - 1): the -1 keeps fp8 exp output < ~100 (e4m3 max 240)
                    nc.scalar.activation(es_sb[s][:, jt, :], ps, AF.Exp,
                                         bias=neg1, scale=SCALE * INV_WS)
                    # den[i] += sum_j(this jt-pair): tiny DR matmuls into a memset
                    # psum bank fill PE bubbles during the S^T phase
                    if jt % 2 == 1:
                        for ic in range(NT):
                            nc.tensor.matmul(
                                den_ps[:, ic:ic + 1],
                                lhsT=es_sb[s][:, jt - 1:jt + 1,
                                              ic * 128:(ic + 1) * 128],
                                rhs=ones8,
                                start=False, stop=False, skip_group_check=True,
                                perf_mode=DR)
                return den_ps

            # recip is a separate phase: its DVE instruction waits on the whole
            # den accumulation, so emitting it late keeps the in-order DVE queue
            # from blocking unrelated work (gn/copies) behind it.
            def emit_recip(s, den_ps):
                recip[s] = statp.tile([128, NT], F32, tag="recip", name=f"recip{s}")
                nc.vector.reciprocal(recip[s], den_ps)

            # ---------------- AV (-> oT[i, c], scaled by 1/den) ----------------
            def emit_av(s):
                oT_sb[s] = otp.tile([128, NT, C], BF16, tag="oT", name=f"oT{s}")
                # residual input x' = x + c0, in place; DVE runs this while the
                # PE streams the AV matmuls
                for ct in range(CT):
                    nc.vector.tensor_scalar(x_sb[s][:, ct, :], x_sb[s][:, ct, :],
                                            c0[:, ct:ct + 1], None, OP.add)
                for it in range(NT):
                    ps = ps_mid.tile([128, 512], F32, tag="mid")
                    for jp2 in range(NT // 2):
                        nc.tensor.matmul(ps,
                                         lhsT=es_sb[s][:, 2 * jp2:2 * jp2 + 2,
                                                       it * 128:(it + 1) * 128],
                                         rhs=vT_sb[s][:, 2 * jp2:2 * jp2 + 2, :],
                                         start=(jp2 == 0), stop=(jp2 == NT // 2 - 1),
                                         perf_mode=DR)
                    nc.vector.tensor_scalar(oT_sb[s][:, it, :], ps, recip[s][:, it:it + 1],
                                            None, OP.mult)

            # ------- transpose oT -> [c, n]; y = oT^T/WS + (x + c0); DMA out -------
            def emit_out(s):
                fin_sb[s] = finp.tile([128, CT, N], BF16, tag="fin", name=f"fin{s}")
                for ct in range(CT):
                    ps = ps_big.tile([128, N], BF16, tag="big", name=f"tp{s}_{ct}")
                    for it in range(NT):
                        nc.tensor.transpose(ps[:, it * 128:(it + 1) * 128],
                                            oT_sb[s][:, it, ct * 128:(ct + 1) * 128],
                                            ident_bf)
                    nc.vector.scalar_tensor_tensor(fin_sb[s][:, ct, :], ps, INV_WS,
                                                   x_sb[s][:, ct, :], OP.mult, OP.add)
                    eng = nc.sync if ct % 2 == 0 else nc.scalar
                    eng.dma_start(out_d[s, ct * 128:(ct + 1) * 128, :],
                                  fin_sb[s][:, ct, :])

            # Phase order keeps PE streaming across the spots where it would
            # otherwise wait on ACT (exp) or DVE (copies/scales).
            emit_gn(0)
            emit_uv(0)
            den0 = emit_s(0)
            emit_gn(1)
            emit_uv(1)
            emit_recip(0, den0)
            emit_av(0)
            den1 = emit_s(1)
            emit_out(0)
            emit_recip(1, den1)
            emit_av(1)
            emit_out(1)

    nc.finalize()
    return nc


_NC_CACHE = None
LAST_EXEC_NS = None
LAST_RESULTS = None


def _get_nc():
    global _NC_CACHE
    if _NC_CACHE is None:
        _NC_CACHE = build_nc()
    return _NC_CACHE


def make_gmat():
    g = np.zeros((128, 128), np.float32)
    g[:64, :64] = 1.0 / 64
    g[64:, 64:] = 1.0 / 64
    return g


def make_in_maps(x, norm_w, norm_b, qkv_w, qkv_b, proj_w, proj_b):
    f8 = ml_dtypes.float8_e4m3
    x = np.asarray(x, np.float32)
    B = x.shape[0]
    x_r = np.ascontiguousarray(x.reshape(B, C, N).astype(ml_dtypes.bfloat16))
    qw = np.asarray(qkv_w, np.float32)    # [3C, C] = [out, in]
    pw = np.asarray(proj_w, np.float32)   # [C, C]
    qb = np.asarray(qkv_b, np.float32)    # [3C]
    # S = h_j . (M^T h_i + r): M = Wq^T Wk, r = Wk^T bq  (bk cancels in softmax)
    smat = qw[0:C].T @ qw[C:2 * C]                      # [c_in, c_out]
    ub = qw[C:2 * C].T @ qb[0:C]                        # [C]
    # proj folded into v: v' = (Wp Wv) h; c0 = Wp bv + pb
    wpv = pw @ qw[2 * C:3 * C]                          # [out, in]
    c0 = pw @ qb[2 * C:3 * C] + np.asarray(proj_b, np.float32)
    common = {
        "smat": np.ascontiguousarray(smat * WS).astype(f8),
        "vwT": np.ascontiguousarray(wpv.T * WS).astype(f8),
        "norm_w": np.ascontiguousarray(np.asarray(norm_w, np.float32)),
        "norm_b": np.ascontiguousarray(np.asarray(norm_b, np.float32)),
        "ub": np.ascontiguousarray(ub * WS),
        "c0": np.ascontiguousarray(c0),
        "gmat": make_gmat(),
    }
    per = B // NCORES
    return [dict(common, x=np.ascontiguousarray(x_r[c * per:(c + 1) * per]))
            for c in range(NCORES)]


def kernel(x, norm_w, norm_b, qkv_w, qkv_b, proj_w, proj_b, _trace=False):
    global LAST_EXEC_NS, LAST_RESULTS
    x = np.asarray(x)
    B, C_, H, W = x.shape
    in_maps = make_in_maps(x, norm_w, norm_b, qkv_w, qkv_b, proj_w, proj_b)
    res = run_bass_kernel_spmd(_get_nc(), in_maps, core_ids=list(range(NCORES)),
                               trace=_trace)
    LAST_EXEC_NS = res.exec_time_ns
    LAST_RESULTS = res
    out = np.concatenate([res.results[c]["out"] for c in range(NCORES)], axis=0)
    return out.reshape(B, C_, H, W).astype(np.float32)
